# revision 1
# baseline (speedup 1.0000x reference)
"""MoE feed-forward (top-1 routing) Trainium2 kernel.

Strategy
--------
Pass 1 (device, 8 cores, token-parallel): gate logits^T = Wg^T @ X^T + bg
  in exact fp32 (argmax margins can be ~3e-5, so the gate matmul must be
  fp32-accurate).
Host: argmax -> expert index per token; group token ids by expert; each
  expert's token list is split across 2 cores (8 cores = 4 experts x 2).
  Pure data movement (gather columns of X^T, cast to fp16).
Pass 2 (device, 8 cores, expert-parallel): per core, the routed tokens'
  FFN for one expert: Y^T = W2^T @ gelu(W1^T @ X^T + b1) + b2, fp16
  operands with fp32 PSUM accumulation (PE runs fp16 at 4x the fp32 rate).
Host: scatter rows back into the [B, L, D] output.
"""

import sys

if "/opt/trn_rl_repo" not in sys.path:
    sys.path.insert(0, "/opt/trn_rl_repo")

import numpy as np

import concourse.bacc as bacc
import concourse.mybir as mybir
import concourse.tile as tile

D, F, E = 1024, 4096, 4
B, L = 4, 2048
T = B * L
NC = 8
TPC = T // NC  # tokens per core in the gate pass
P = 128
KD = D // P    # 8  k-tiles over D
KF = F // P    # 32 k-tiles over F

TRACE = False
LAST_EXEC_NS = []
LAST_TRACES = []

_cache = {}


def _run(nc, in_maps):
    from concourse import bass_utils

    if TRACE:
        bass_utils.upload_artifacts = lambda d: "local://" + d
    res = bass_utils.run_bass_kernel_spmd(
        nc, in_maps, core_ids=list(range(NC)), trace=TRACE
    )
    if TRACE:
        LAST_EXEC_NS.append(res.exec_time_ns)
        LAST_TRACES.append(
            res.instructions_and_trace[1] if res.instructions_and_trace else None
        )
    return res


def _subchunks(C):
    subs = []
    s = 0
    while s < C:
        sz = min(512, C - s)
        subs.append((s, sz))
        s += sz
    return subs


def _build_gate():
    if "gate" in _cache:
        return _cache["gate"]
    f32 = mybir.dt.float32
    nc = bacc.Bacc("TRN2", target_bir_lowering=False, debug=False, num_devices=NC)
    xt = nc.dram_tensor("xt", (D, TPC), f32, kind="ExternalInput")
    wg = nc.dram_tensor("wg", (D, E), f32, kind="ExternalInput")
    bg = nc.dram_tensor("bg", (E, 1), f32, kind="ExternalInput")
    lo = nc.dram_tensor("lo", (E, TPC), f32, kind="ExternalOutput")

    with tile.TileContext(nc) as tc:
        with (
            tc.tile_pool(name="sbuf", bufs=1) as pool,
            tc.tile_pool(name="psum", bufs=2, space="PSUM") as psum,
        ):
            wgt = pool.tile([P, KD, E], f32)
            nc.sync.dma_start(wgt[:], wg.ap().rearrange("(ko p) e -> p ko e", p=P))
            bgt = pool.tile([E, 1], f32)
            nc.sync.dma_start(bgt[:], bg.ap()[:])
            xts = pool.tile([P, KD, TPC], f32)
            for k in range(KD):
                nc.sync.dma_start(xts[:, k], xt.ap()[k * P:(k + 1) * P, :])
            for (s0, sz) in _subchunks(TPC):
                pt = psum.tile([E, 512], f32, name="pg")
                for k in range(KD):
                    nc.tensor.matmul(
                        pt[:, :sz], wgt[:, k], xts[:, k, s0:s0 + sz],
                        start=(k == 0), stop=(k == KD - 1),
                    )
                ls = pool.tile([E, 512], f32, name="ls")
                nc.vector.tensor_scalar_add(ls[:, :sz], pt[:, :sz], bgt[:, 0:1])
                nc.sync.dma_start(lo.ap()[:, s0:s0 + sz], ls[:, :sz])
    nc.compile()
    _cache["gate"] = nc
    return nc


def _build_ffn(C):
    key = ("ffn", C)
    if key in _cache:
        return _cache[key]
    f32 = mybir.dt.float32
    f16 = mybir.dt.float16
    subs = _subchunks(C)
    nc = bacc.Bacc("TRN2", target_bir_lowering=False, debug=False, num_devices=NC)
    xt = nc.dram_tensor("xt", (D, C), f16, kind="ExternalInput")
    w1 = nc.dram_tensor("w1", (KF, P, KD, P), f16, kind="ExternalInput")
    b1 = nc.dram_tensor("b1", (P, KF), f32, kind="ExternalInput")
    w2 = nc.dram_tensor("w2", (KD, P, KF, P), f16, kind="ExternalInput")
    b2 = nc.dram_tensor("b2", (P, KD), f32, kind="ExternalInput")
    yt = nc.dram_tensor("yt", (D, C), f32, kind="ExternalOutput")

    with tile.TileContext(nc) as tc:
        with (
            tc.tile_pool(name="xs", bufs=1) as xpool,
            tc.tile_pool(name="hs", bufs=1) as hpool,
            tc.tile_pool(name="w1p", bufs=3) as w1pool,
            tc.tile_pool(name="w2p", bufs=2) as w2pool,
            tc.tile_pool(name="yp", bufs=3) as ypool,
            tc.tile_pool(name="bp", bufs=1) as bpool,
        ):
            b1t = bpool.tile([P, KF], f32)
            nc.sync.dma_start(b1t[:], b1.ap()[:])
            b2t = bpool.tile([P, KD], f32)
            nc.sync.dma_start(b2t[:], b2.ap()[:])
            xts = xpool.tile([P, KD, C], f16)
            for k in range(KD):
                nc.sync.dma_start(xts[:, k], xt.ap()[k * P:(k + 1) * P, :])

            hts = [hpool.tile([P, C], f16, name=f"h{f}") for f in range(KF)]

            # ---- layer 1: H^T[f] = gelu(sum_k W1[k,f]^T X^T[k] + b1[f]) ----
            with tc.tile_pool(name="ps1", bufs=2, space="PSUM") as psum1:
                for f in range(KF):
                    w1t = w1pool.tile([P, KD, P], f16, name="w1t")
                    nc.sync.dma_start(w1t[:], w1.ap()[f])
                    pts = [
                        psum1.tile([P, 512], f32, name=f"p1_{si}")
                        for si in range(len(subs))
                    ]
                    for k in range(KD):
                        for si, (s0, sz) in enumerate(subs):
                            nc.tensor.matmul(
                                pts[si][:, :sz], w1t[:, k], xts[:, k, s0:s0 + sz],
                                start=(k == 0), stop=(k == KD - 1),
                            )
                    for si, (s0, sz) in enumerate(subs):
                        nc.scalar.activation(
                            hts[f][:, s0:s0 + sz], pts[si][:, :sz],
                            mybir.ActivationFunctionType.Gelu,
                            bias=b1t[:, f:f + 1], scale=1.0,
                        )

            # ---- layer 2: Y^T[d] = sum_f W2[f,d]^T H^T[f] + b2[d] ----
            with tc.tile_pool(name="ps2", bufs=2, space="PSUM") as psum2:
                for d in range(KD):
                    w2t = w2pool.tile([P, KF, P], f16, name="w2t")
                    nc.sync.dma_start(w2t[:], w2.ap()[d])
                    pts = [
                        psum2.tile([P, 512], f32, name=f"p2_{si}")
                        for si in range(len(subs))
                    ]
                    for f in range(KF):
                        for si, (s0, sz) in enumerate(subs):
                            nc.tensor.matmul(
                                pts[si][:, :sz], w2t[:, f], hts[f][:, s0:s0 + sz],
                                start=(f == 0), stop=(f == KF - 1),
                            )
                    for si, (s0, sz) in enumerate(subs):
                        ys = ypool.tile([P, 512], f32, name="ysb")
                        nc.vector.tensor_scalar_add(
                            ys[:, :sz], pts[si][:, :sz], b2t[:, d:d + 1]
                        )
                        nc.sync.dma_start(
                            yt.ap()[d * P:(d + 1) * P, s0:s0 + sz], ys[:, :sz]
                        )
    nc.compile()
    _cache[key] = nc
    return nc


def kernel(x, W1, b1, W2, b2, Wg, bg):
    x = np.asarray(x, dtype=np.float32)
    W1 = np.asarray(W1, dtype=np.float32)
    b1 = np.asarray(b1, dtype=np.float32)
    W2 = np.asarray(W2, dtype=np.float32)
    b2 = np.asarray(b2, dtype=np.float32)
    Wg = np.asarray(Wg, dtype=np.float32)
    bg = np.asarray(bg, dtype=np.float32)

    xf = x.reshape(T, D)
    XT = np.ascontiguousarray(xf.T)  # [D, T] f32

    # ---- pass 1: gate logits on device ----
    nc1 = _build_gate()
    in_maps = [
        {
            "xt": np.ascontiguousarray(XT[:, c * TPC:(c + 1) * TPC]),
            "wg": Wg,
            "bg": bg.reshape(E, 1),
        }
        for c in range(NC)
    ]
    res1 = _run(nc1, in_maps)
    logits = np.concatenate(
        [res1.results[c]["lo"].T for c in range(NC)], axis=0
    )  # [T, E]
    idx = np.argmax(logits, axis=1)

    # ---- host routing: group tokens by expert, split each across 2 cores ----
    lists = []
    for e in range(E):
        te = np.nonzero(idx == e)[0]
        h0 = (len(te) + 1) // 2
        lists.append(te[:h0])
        lists.append(te[h0:])
    C = max(1, max(len(l) for l in lists))
    C = ((C + 63) // 64) * 64

    XT16 = XT.astype(np.float16)
    W1d = W1.reshape(E, KD, P, KF, P).transpose(0, 3, 2, 1, 4)  # [E, KF, P, KD, P]
    W1d = np.ascontiguousarray(W1d).astype(np.float16)
    W2d = W2.reshape(E, KF, P, KD, P).transpose(0, 3, 2, 1, 4)  # [E, KD, P, KF, P]
    W2d = np.ascontiguousarray(W2d).astype(np.float16)
    b1d = np.ascontiguousarray(b1.reshape(E, KF, P).transpose(0, 2, 1))  # [E, P, KF]
    b2d = np.ascontiguousarray(b2.reshape(E, KD, P).transpose(0, 2, 1))  # [E, P, KD]

    in_maps2 = []
    for c in range(NC):
        e = c // 2
        l = lists[c]
        tok = np.zeros(C, dtype=np.int64)
        tok[:len(l)] = l
        in_maps2.append(
            {
                "xt": np.ascontiguousarray(XT16[:, tok]),
                "w1": W1d[e],
                "b1": b1d[e],
                "w2": W2d[e],
                "b2": b2d[e],
            }
        )

    nc2 = _build_ffn(C)
    res2 = _run(nc2, in_maps2)

    out = np.empty((T, D), dtype=np.float32)
    for c in range(NC):
        l = lists[c]
        if len(l):
            out[l] = res2.results[c]["yt"][:, :len(l)].T
    return out.reshape(B, L, D)
